# revision 34
# baseline (speedup 1.0000x reference)
"""Trainium2 Bass kernel for Transformer-XL relative multi-head attention.

Problem: nn_MultiHeadAttn_27290222199184
  T=1024 queries, MEM=1024 memory, C=2048 keys, B=4, DM=1024, N=16 heads, D=64.

Sharding (8 NeuronCores, SPMD — one program, per-core data slices):
  core = 2*b + nh   (b in 0..3 batch, nh in 0..1 head-half)
  Each core computes attention for batch b over its 8 heads and emits the
  partial output projection vec @ W_o[nd_half] -> [T, DM].
  Host: sums the two half-partials per batch, adds residual h, layernorm.

v2 design (cost-model driven):
  - host pre-transposes cat -> catT and precomputes rk = r @ W_r (batch-
    independent); all matmul operands bf16.
  - causal mask folded into the BD scratch tails: tails hold -70000 so the
    skewed rel-shift read delivers masked scores for free (no mask tensor,
    no copy_predicated).
  - BD rel-shift round trip in bf16 with ONE dram write + ONE skewed read
    per (head, i-tile).
  - S^T is formed by PE transposes of the S stash; exp reads S^T straight
    from PSUM and writes PT to SBUF (no separate PSUM->SBUF copy for P).
  - softmax denominators via a ones-column appended to V (AV output col 64),
    reciprocal + scale applied per-partition on the AV output [i, d]; vec is
    then pair-transposed once per (pair, i-tile) for the Wo projection.
"""

import sys
from contextlib import ExitStack

if "/opt/trn_rl_repo" not in sys.path:
    sys.path.insert(0, "/opt/trn_rl_repo")

import numpy as np

import concourse.bass as bass
import concourse.bacc as bacc
import concourse.tile as tile
from concourse import mybir

T, MEM, B, DM, N, D = 1024, 1024, 4, 1024, 16, 64
C = MEM + T
NH = N // 2          # heads per core
NP = NH // 2         # head pairs per core
SCALE = 1.0 / D ** 0.5
LN_EPS = 1e-5

BDW = 2560           # bd scratch row width (elements)
NBD = 16             # bd scratch buffers
NEG = -70000.0

F32 = mybir.dt.float32
BF16 = mybir.dt.bfloat16

ADD = mybir.AluOpType.add
MULT = mybir.AluOpType.mult


def _mlo(it):
    """first bd column needed by i-tile it."""
    return max(0, (T - 1) - it * 128 - 127)


def _cmax(it):
    """last score 512-chunk containing any unmasked element for i-tile it."""
    return (it * 128 + 127 + MEM) // 512


def build_nc():
    nc = bacc.Bacc("TRN2", target_bir_lowering=False, debug=False)

    io = {}
    io["catT"] = nc.dram_tensor("catT", [DM, C], BF16, kind="ExternalInput")
    io["rk_p"] = nc.dram_tensor("rk_p", [128, NP * C], BF16, kind="ExternalInput")
    for w in ("Wq", "Wk"):
        io[w] = nc.dram_tensor(w, [DM, NH * D], BF16, kind="ExternalInput")
    io["Wv"] = nc.dram_tensor("Wv", [DM, NH * D], BF16, kind="ExternalInput")
    io["Wo"] = nc.dram_tensor("Wo", [NH * D, DM], BF16, kind="ExternalInput")
    io["ident"] = nc.dram_tensor("ident", [128, 128], BF16, kind="ExternalInput")
    io["rwb_p"] = nc.dram_tensor("rwb_p", [128, NP], F32, kind="ExternalInput")
    io["rrb_p"] = nc.dram_tensor("rrb_p", [128, NP], F32, kind="ExternalInput")
    io["out"] = nc.dram_tensor("out", [T, DM], F32, kind="ExternalOutput")

    io["bd"] = [nc.dram_tensor(f"bd_s{i}", [128, BDW], BF16) for i in range(NBD)]

    with tile.TileContext(nc) as tc:
        _emit(nc, tc, io)
    nc.compile()
    return nc


def _emit(nc, tc, io):
    ctx = ExitStack()
    with ctx:
        singles = ctx.enter_context(tc.tile_pool(name="singles", bufs=1))
        resid = ctx.enter_context(tc.tile_pool(name="resid", bufs=1))
        catT_p = ctx.enter_context(tc.tile_pool(name="catT", bufs=1))
        wset_p = ctx.enter_context(tc.tile_pool(name="wset", bufs=2))
        stash_p = ctx.enter_context(tc.tile_pool(name="stash", bufs=2))
        pt_p = ctx.enter_context(tc.tile_pool(name="pt", bufs=2))
        bdst_p = ctx.enter_context(tc.tile_pool(name="bdst", bufs=5))
        st_p = ctx.enter_context(tc.tile_pool(name="st", bufs=2))
        vec_p = ctx.enter_context(tc.tile_pool(name="vec", bufs=2))
        rc_p = ctx.enter_context(tc.tile_pool(name="rc", bufs=4))
        wo_p = ctx.enter_context(tc.tile_pool(name="wo", bufs=1))

        psum_mm = ctx.enter_context(tc.tile_pool(name="psum_mm", bufs=4, space="PSUM"))
        psum_tp = ctx.enter_context(tc.tile_pool(name="psum_tp", bufs=2, space="PSUM"))
        psum_av = ctx.enter_context(tc.tile_pool(name="psum_av", bufs=2, space="PSUM"))

        # ---------------- constants / residents ----------------
        # load order follows first use: catT+Wq gate everything, then rk
        # (BD prologue), Wv, ident (phase-B transposes), bd tail fills
        # (first skew read).
        rwb_t = singles.tile([128, NP], F32)
        nc.sync.dma_start(rwb_t, io["rwb_p"].ap())
        rrb_t = singles.tile([128, NP], F32)
        nc.sync.dma_start(rrb_t, io["rrb_p"].ap())

        kres = resid.tile([128, NP, C], BF16)       # pair-packed k^T
        rkres = resid.tile([128, NP, C], BF16)      # pair-packed (r@Wr)^T
        vaug = resid.tile([128, 16, NH, 65], BF16)  # v + ones column
        qbT = resid.tile([128, NP, T], BF16)
        q2T = resid.tile([128, NP, T], BF16)
        vecT = resid.tile([128, NP, T], BF16)
        ident = singles.tile([128, 128], BF16)
        negf = singles.tile([128, 512], BF16)
        wv_t = singles.tile([128, 8, 512], BF16)

        def emit_late_loads():
            nc.sync.dma_start(
                rkres, io["rk_p"].ap().rearrange("pp (p c) -> pp p c", p=NP))
            nc.sync.dma_start(
                wv_t, io["Wv"].ap().rearrange("(o pp) n -> pp o n", pp=128))
            nc.sync.dma_start(ident, io["ident"].ap())
            nc.vector.memset(vaug[:, :, :, 64:65], 1.0)
            # bd tails [2048, BDW) are read by the skew reads exactly at
            # masked positions (m = 1023+j-i >= 2048 <=> j > i+MEM): fill
            # once with NEG so masking is free.
            nc.vector.memset(negf, NEG)
            for buf in io["bd"]:
                nc.sync.dma_start(buf.ap()[:, 2048:2560], negf)

        def load_wset(wname, p):
            ws = wset_p.tile([128, 8, 128], BF16, tag="wset")
            nc.sync.dma_start(
                ws,
                io[wname].ap()[:, p * 128:(p + 1) * 128].rearrange(
                    "(o pp) n -> pp o n", pp=128),
            )
            return ws

        def emit_bd(hh, its):
            """BD raw band matmuls -> staging -> dram ring, for i-tiles its."""
            p_, sub_ = hh // 2, hh % 2
            lo_, hi_ = 64 * sub_, 64 * sub_ + 64
            for it in its:
                buf = io["bd"][(hh * 8 + it) % NBD]
                mlo = _mlo(it)
                bst = bdst_p.tile([128, 2048], BF16, tag="bdst")
                for a in range(mlo // 512, 4):
                    off = max(mlo, 512 * a)
                    w = 512 * (a + 1) - off
                    ps = psum_mm.tile([128, 512], F32, tag="mm")
                    nc.tensor.matmul(
                        ps[:, :w],
                        (q2T[lo_:hi_, p_, it * 128:(it + 1) * 128]),
                        (rkres[lo_:hi_, p_, off:off + w]),
                        start=True, stop=True,
                    )
                    if (it + a) % 3 != 0:
                        nc.scalar.copy(bst[:, off:off + w], ps[:, :w])
                    else:
                        nc.vector.tensor_copy(bst[:, off:off + w], ps[:, :w])
                nc.sync.dma_start(buf.ap()[:, mlo:2048], bst[:, mlo:2048])

        def pha_kt(cat2, half):
            for p in range(NP):
                ws = load_wset("Wk", p)
                for ch in range(2):
                    cc512 = half * 2 + ch
                    ps = psum_mm.tile([128, 512], F32, tag="mm")
                    for dmc in range(8):
                        nc.tensor.matmul(
                            ps, (ws[:, dmc, :]), (cat2[ch][:, dmc, :]),
                            start=(dmc == 0), stop=(dmc == 7),
                        )
                    nc.scalar.copy(kres[:, p, cc512 * 512:(cc512 + 1) * 512], ps)

        def pha_v(cat2, half):
            for cc in range(8):
                ps = psum_mm.tile([128, 512], F32, tag="mm")
                for dmc in range(8):
                    nc.tensor.matmul(
                        ps, (cat2[cc // 4][:, dmc, (cc % 4) * 128:(cc % 4 + 1) * 128]),
                        (wv_t[:, dmc, :]),
                        start=(dmc == 0), stop=(dmc == 7),
                    )
                nc.scalar.copy(
                    vaug[:, half * 8 + cc, :, 0:64],
                    ps.rearrange("pp (h d) -> pp h d", h=NH),
                )

        def load_cat(half):
            tiles = []
            for sh in range(2):
                t = catT_p.tile([128, 8, 512], BF16, tag=f"catT{sh}")
                c0 = half * 1024 + sh * 512
                nc.sync.dma_start(
                    t, io["catT"].ap()[:, c0:c0 + 512].rearrange(
                        "(o pp) c -> pp o c", pp=128))
                tiles.append(t)
            return tiles

        # half 1 first: q projections unblock the BD prologue
        cat1 = load_cat(1)
        for ih in range(2):
            for p in range(NP):
                ws = load_wset("Wq", p)
                ps = psum_mm.tile([128, 512], F32, tag="mm")
                for dmc in range(8):
                    nc.tensor.matmul(
                        ps, (ws[:, dmc, :]), (cat1[ih][:, dmc, :]),
                        start=(dmc == 0), stop=(dmc == 7),
                    )
                # biases arrive pre-scaled by SCALE from the host
                nc.vector.tensor_scalar(
                    qbT[:, p, ih * 512:(ih + 1) * 512], ps,
                    SCALE, rwb_t[:, p:p + 1], MULT, ADD)
                nc.vector.tensor_scalar(
                    q2T[:, p, ih * 512:(ih + 1) * 512], ps,
                    SCALE, rrb_t[:, p:p + 1], MULT, ADD)
        emit_late_loads()
        pha_kt(cat1, 1)
        # BD prologue overlaps the rest of phase A
        emit_bd(0, range(4))
        pha_v(cat1, 1)
        emit_bd(0, range(4, 8))
        cat0 = load_cat(0)
        pha_kt(cat0, 0)
        emit_bd(1, range(4))
        pha_v(cat0, 0)
        emit_bd(1, range(4, 8))

        # ------------- phase B: attention -------------
        for p in range(NP):
            vecp = vec_p.tile([128, 8, 128], BF16, tag="vecp")
            for sub in range(2):
                hh = 2 * p + sub
                lo, hi = 64 * sub, 64 * sub + 64

                for ihalf in range(2):
                    W = 512 * (_cmax(ihalf * 4) + 1)     # 1536 or 2048
                    njb = W // 128                        # 12 or 16 j-blocks
                    stash = stash_p.tile([128, 4, 2048], BF16, tag="stash")
                    ptt = pt_p.tile([128, 16, 512], BF16, tag="pt")
                    # skewed BD reads land directly in the stash rows
                    for itl in range(4):
                        it = ihalf * 4 + itl
                        buf = io["bd"][(hh * 8 + it) % NBD]
                        nc.sync.dma_start(
                            stash[:, itl, :W],
                            bass.AP(buf, (T - 1) - it * 128, [[BDW - 1, 128], [1, W]]),
                        )
                    # AC chunks + in-place adds (c-major so transposes unblock early)
                    for c in range(W // 512):
                        for itl in range(4):
                            it = ihalf * 4 + itl
                            ps = psum_mm.tile([128, 512], F32, tag="mm")
                            nc.tensor.matmul(
                                ps,
                                (qbT[lo:hi, p, it * 128:(it + 1) * 128]),
                                (kres[lo:hi, p, c * 512:(c + 1) * 512]),
                                start=True, stop=True,
                            )
                            nc.vector.tensor_tensor(
                                stash[:, itl, c * 512:(c + 1) * 512],
                                stash[:, itl, c * 512:(c + 1) * 512], ps, ADD)
                    # fill PE stalls: next-next head's BD pass for this half
                    if hh + 2 < NH:
                        emit_bd(hh + 2, range(ihalf * 4, ihalf * 4 + 4))
                    # S^T blocks -> exp -> PT -> AV, pipelined per jb2
                    av4 = psum_av.tile([128, 4, 65], F32, tag="av")
                    for jb2 in range(njb // 2):
                        tps = psum_tp.tile([128, 2, 512], BF16, tag="tp")
                        for k in range(2):
                            jb = jb2 * 2 + k
                            for itl in range(4):
                                nc.tensor.transpose(
                                    (tps[:, k, itl * 128:(itl + 1) * 128]),
                                    (stash[:, itl, jb * 128:(jb + 1) * 128]),
                                    (ident),
                                )
                        nc.scalar.activation(
                            ptt[:, jb2 * 2:jb2 * 2 + 2, :], tps,
                            mybir.ActivationFunctionType.Exp)
                        for itl in range(4):
                            for k in range(2):
                                jb = jb2 * 2 + k
                                nc.tensor.matmul(
                                    av4[:, itl, :],
                                    (ptt[:, jb, itl * 128:(itl + 1) * 128]),
                                    (vaug[:, jb, hh, :]),
                                    start=(jb == 0), stop=(jb == njb - 1),
                                )
                    for itl in range(4):
                        it = ihalf * 4 + itl
                        recip = rc_p.tile([128, 1], F32, tag="rc")
                        nc.vector.reciprocal(recip, av4[:, itl, 64:65])
                        nc.vector.tensor_scalar(
                            vecp[:, it, lo:hi], av4[:, itl, 0:64], recip, None, MULT)

            # pair done: transpose vec [i, nd] -> vecT [nd, i]
            for it in range(8):
                tps = psum_tp.tile([128, 512], BF16, tag="tp")
                nc.tensor.transpose((tps[:, 0:128]), (vecp[:, it, :]), (ident))
                nc.vector.tensor_copy(vecT[:, p, it * 128:(it + 1) * 128], tps[:, 0:128])

        # ------------- phase C: output projection -------------
        for dmc in range(2):
            wot = wo_p.tile([128, NP, 512], BF16, tag="wo")
            nc.sync.dma_start(
                wot,
                io["Wo"].ap()[:, dmc * 512:(dmc + 1) * 512].rearrange(
                    "(p pp) d -> pp p d", pp=128),
            )
            for it in range(8):
                ps = psum_mm.tile([128, 512], F32, tag="mm")
                for pp in range(NP):
                    nc.tensor.matmul(
                        ps, (vecT[:, pp, it * 128:(it + 1) * 128]), (wot[:, pp, :]),
                        start=(pp == 0), stop=(pp == NP - 1),
                    )
                st = st_p.tile([128, 512], F32, tag="st")
                nc.scalar.copy(st, ps)
                nc.sync.dma_start(
                    io["out"].ap()[it * 128:(it + 1) * 128, dmc * 512:(dmc + 1) * 512], st)


_NC = None


def _get_nc():
    global _NC
    if _NC is None:
        _NC = build_nc()
    return _NC


def _bf16(x):
    import ml_dtypes
    return np.ascontiguousarray(np.asarray(x, dtype=ml_dtypes.bfloat16))


def make_in_maps(h, m, r, mask, W_qkv, W_r, W_o, r_w_bias, r_r_bias):
    h = np.asarray(h, dtype=np.float32)
    m = np.asarray(m, dtype=np.float32)
    r = np.asarray(r, dtype=np.float32)
    W_qkv = np.asarray(W_qkv, dtype=np.float32)
    W_r = np.asarray(W_r, dtype=np.float32)
    W_o = np.asarray(W_o, dtype=np.float32)
    rwb = np.asarray(r_w_bias, dtype=np.float32)
    rrb = np.asarray(r_r_bias, dtype=np.float32)

    rk_full = r @ W_r                     # [C, N*D], batch-independent
    ident = np.eye(128, dtype=np.float32)

    in_maps = []
    for core in range(8):
        b, nh = core // 2, core % 2
        sl = slice(nh * NH * D, (nh + 1) * NH * D)
        rwb_p = np.zeros((128, NP), np.float32)
        rrb_p = np.zeros((128, NP), np.float32)
        rk_p = np.zeros((128, NP * C), np.float32)
        rk_sl = rk_full[:, sl]            # [C, NH*D]
        for hh in range(NH):
            g = nh * NH + hh
            rows = slice(64 * (hh % 2), 64 * (hh % 2) + 64)
            pcol = hh // 2
            rwb_p[rows, pcol] = rwb[g] * SCALE
            rrb_p[rows, pcol] = rrb[g] * SCALE
            rk_p[rows, pcol * C:(pcol + 1) * C] = rk_sl[:, hh * 64:(hh + 1) * 64].T
        cat = np.concatenate([m[:, b, :], h[:, b, :]], axis=0)  # [C, DM]
        in_maps.append({
            "catT": _bf16(cat.T),
            "rk_p": _bf16(rk_p),
            "Wq": _bf16(W_qkv[:, 0 * N * D:1 * N * D][:, sl]),
            "Wk": _bf16(W_qkv[:, 1 * N * D:2 * N * D][:, sl]),
            "Wv": _bf16(W_qkv[:, 2 * N * D:3 * N * D][:, sl]),
            "Wo": _bf16(W_o[sl, :]),
            "rwb_p": rwb_p,
            "rrb_p": rrb_p,
            "ident": _bf16(ident),
        })
    return in_maps


def finish(h, parts, ln_gamma, ln_beta):
    h = np.asarray(h, dtype=np.float32)
    gamma = np.asarray(ln_gamma, dtype=np.float32)
    beta = np.asarray(ln_beta, dtype=np.float32)
    out = np.empty((T, B, DM), np.float32)
    for b in range(B):
        x = h[:, b, :] + parts[2 * b] + parts[2 * b + 1]
        mu = x.mean(axis=-1, keepdims=True, dtype=np.float32)
        var = ((x - mu) ** 2).mean(axis=-1, keepdims=True, dtype=np.float32)
        out[:, b, :] = (x - mu) / np.sqrt(var + LN_EPS) * gamma + beta
    return out


def kernel(h, m, r, mask, W_qkv, W_r, W_o, r_w_bias, r_r_bias, ln_gamma, ln_beta):
    from concourse.bass_utils import run_bass_kernel_spmd

    in_maps = make_in_maps(h, m, r, mask, W_qkv, W_r, W_o, r_w_bias, r_r_bias)
    res = run_bass_kernel_spmd(_get_nc(), in_maps, core_ids=list(range(8)))
    parts = [np.asarray(res.results[c]["out"]) for c in range(8)]
    return finish(h, parts, ln_gamma, ln_beta)
